# revision 40
# baseline (speedup 1.0000x reference)
"""Trainium2 Bass kernel for nn_LinearDiffusion (truncated Taylor expm(a) @ x).

Math: a = row-normalized symmetric scatter of per-head edge weights onto an
(H, N, N) zero tensor; result = sum_{i=0..6} a^i x / i! with x = h reshaped
per-head.

Strategy (8 NeuronCores, one chip):
  * Sparse formulation, dst-row sharded: core k owns rows [k*1024,(k+1)*1024).
  * x (8192 x 64, all heads interleaved per node row) lives in SBUF as fp16
    (1 MB); no DMA row-gather at all.  Both the gather x[src] and the
    scatter into dst rows run on the Tensor engine as one-hot fp8 matmuls:
      - per (dst-block I, src-block J) pair one full 128-slot chunk, so every
        stationary is a full 128-col fp8 load (FWL-eligible, no column
        tiling, no PE array mode switches - these were worth 2.2x);
        gather: psG[chunk] = G_pair^T @ x_J
      - VectorE: per-head weight multiply, PSUM -> fp16 SBUF
      - scatter: psY_I += S_chunk^T @ xgw_chunk  (fp8 one-hot stationary)
  * The slot layout is identical on every core, so one SPMD program serves
    all 8 cores; only the table *data* (one-hot columns, weights) differs.
  * PSUM discipline: start=True clears has_written bank-wide, and evac
    reads must not share a bank with in-flight matmul writes, so the y
    accumulators are 4 separate banks: (phase A|B) x (blocks 0-3|4-7).
  * Between the k=6 matmuls: two half fp16 AllGathers (each core's blocks
    0-3, then 4-7).  The pair permutation puts AG-A's src blocks in each
    dst block's first chunks, and waves run phase-major, so AG-A overlaps
    the producer's second half and AG-B overlaps the consumer's first.
  * Software-pipelined wave loop: gathers run LAG=2 waves ahead of
    scatters; table loads are split per (phase, block) in first-use order.
"""

import math
from dataclasses import dataclass

import numpy as np

import concourse.bass as bass  # noqa: F401  (kept for callers)
import concourse.tile as tile
from concourse import bacc, mybir
from concourse.bass_utils import run_bass_kernel_spmd

# ----------------------------------------------------------------- config

N, H, E, D = 8192, 4, 131072, 64
d = D // H
NCORES = 8
BLK = 128  # dst/src block size == PE width
NB = N // BLK  # 64 src blocks
K_TAYLOR = 6


@dataclass(frozen=True)
class Cfg:
    n: int = N
    n_cores: int = NCORES
    hi_lo_split: bool = True  # unused (fp16 path is exact enough); kept for test.py

    @property
    def rows_per_core(self):
        return self.n // self.n_cores

    @property
    def blocks_per_core(self):
        return self.rows_per_core // BLK


# ----------------------------------------------------------- preprocessing


def _entries(e, src, dst, n):
    """Unique symmetric entries with 'last write wins' duplicate semantics,
    matching jax's .at[].set() on CPU. Returns (rows, cols, w[H, nnz])."""
    src = src.astype(np.int64)
    dst = dst.astype(np.int64)
    n_edges = len(src)
    keys = np.concatenate([src * n + dst, dst * n + src])
    eid = np.concatenate([np.arange(n_edges), np.arange(n_edges)])
    order = np.arange(2 * n_edges)
    perm = np.lexsort((-order, keys))
    k_sorted = keys[perm]
    first = np.ones(len(k_sorted), dtype=bool)
    first[1:] = k_sorted[1:] != k_sorted[:-1]
    win = perm[first]
    ukeys = k_sorted[first]
    rows = (ukeys // n).astype(np.int64)
    cols = (ukeys % n).astype(np.int64)
    weids = eid[win]
    vals = e[:, weids].astype(np.float64)  # (H, nnz)
    nheads = e.shape[0]
    rowsum = np.zeros((nheads, n), dtype=np.float64)
    for hh in range(nheads):
        rowsum[hh] = np.bincount(rows, weights=vals[hh], minlength=n)
    w = (vals / rowsum[:, rows]).astype(np.float32)
    return rows, cols, w


# Pair order within each dst block: src blocks delivered by the first
# half-AllGather (j % bpc < bpc/2, i.e. every core's first 4 row-blocks)
# come first, so the next iteration's first chunks depend only on AG-A.
HA = 3  # blocks per core in the first (early) half-AllGather


def _pair_perm():
    return sorted(range(NB), key=lambda j: (j % 8 >= HA, j))


def _pieces_for_ps(ps):
    """Slot layout: pair rank r (in _pair_perm order) owns slots
    [r*ps, (r+1)*ps) of its dst block. ps is a multiple of 128, so each
    chunk belongs to exactly one pair: full-width FWL-eligible matmuls."""
    assert ps % 128 == 0
    perm = _pair_perm()
    cb = (NB * ps) // 128  # chunks per dst block
    cpp = ps // 128  # chunks per pair
    pieces = [[(0, 128, perm[c // cpp], c * 128)] for c in range(cb)]
    return pieces, cb


def _make_tables(e, src, dst, cfg: Cfg):
    """Per-core device tables. Returns (tables, R) where tables is a list
    over cores of dicts with keys gsl, sca (fp8), w4 (fp32)."""
    import ml_dtypes

    n = cfg.n
    rows, cols, w = _entries(e, src, dst, n)
    nheads = w.shape[0]
    bpc = cfg.blocks_per_core

    # global R so the program structure is identical on every core.
    # R is a multiple of 4 => every pair occupies whole 128-slot chunks, so
    # every gather matmul loads a full 128-col stationary (FWL-eligible, no
    # column tiling, no PE array mode switches).
    cnt = np.zeros((n // BLK, NB), dtype=np.int64)
    np.add.at(cnt, (rows // BLK, cols // BLK), 1)
    R = 4 * int(np.ceil(cnt.max() / 128))
    ps = 32 * R
    cb = (NB * ps) // 128  # chunks per dst block
    nch = bpc * cb
    nslots = nch * 128

    tables = []
    for k in range(cfg.n_cores):
        sel = (rows >= k * 1024) & (rows < (k + 1) * 1024)
        r_k, c_k, w_k = rows[sel], cols[sel], w[:, sel]
        rank = np.empty(NB, dtype=np.int64)
        rank[np.array(_pair_perm())] = np.arange(NB)
        b_loc = r_k // BLK - k * bpc  # 0..7
        key = b_loc * NB + rank[c_k // BLK]
        order = np.argsort(key, kind="stable")
        r_k, c_k, w_k, key = r_k[order], c_k[order], w_k[:, order], key[order]
        grp_start = np.searchsorted(key, key)  # first index of each group
        off_in_pair = np.arange(len(key)) - grp_start
        assert off_in_pair.max(initial=0) < ps
        slot = key * ps + off_in_pair  # key = b_loc * NB + pair_rank

        gsl = np.zeros((128, nslots), dtype=ml_dtypes.float8_e4m3fn)
        sca = np.zeros((128, nslots), dtype=ml_dtypes.float8_e4m3fn)
        w4 = np.zeros((128, nch, nheads), dtype=np.float32)
        gsl[c_k % BLK, slot] = 1.0
        sca[slot % 128, (slot // 128) * 128 + (r_k % BLK)] = 1.0
        w4[slot % 128, slot // 128, :] = w_k.T
        tables.append(
            {
                "gsl": gsl,
                "sca": sca,
                "w4": np.ascontiguousarray(w4.reshape(128, nch * nheads)),
                "w4h": np.ascontiguousarray(
                    w4.reshape(128, nch * nheads).astype(np.float16)
                ),
            }
        )
    return tables, R


# ------------------------------------------------------------ bass program

_FP32 = mybir.dt.float32
_FP16 = mybir.dt.float16
_FP8 = mybir.dt.float8e4


def _build_program(cfg: Cfg, R: int):
    bpc = cfg.blocks_per_core
    rpc = cfg.rows_per_core
    ps = 32 * R
    pieces, cb = _pieces_for_ps(ps)
    nch = bpc * cb
    nslots = nch * 128

    WCH = 8  # chunks per wave (one wave = 1 PSUM bank of gathered rows)
    LAG = 2  # scatter waves trail gather waves by this many steps
    assert cb % (2 * WCH) == 0
    waves_pb = cb // WCH  # waves per dst block
    wppA = (NB // bpc * HA * ps) // (128 * WCH)  # phase-A waves per block
    wppB = waves_pb - wppA
    hb = HA  # blocks in AG-A; bpc - HA in AG-B

    nc = bacc.Bacc(
        "TRN2",
        target_bir_lowering=False,
        debug=False,
        num_devices=cfg.n_cores,
    )

    # partition-major x layouts: [core, p, b, f] so SBUF loads are contiguous
    xin16 = nc.dram_tensor("xin16", [128, NB * D], _FP16, kind="ExternalInput").ap()
    x0s_d = nc.dram_tensor("x0s", [rpc, D], _FP32, kind="ExternalInput").ap()
    gsl_d = nc.dram_tensor("gsl", [128, nslots], _FP8, kind="ExternalInput").ap()
    sca_d = nc.dram_tensor("sca", [128, nslots], _FP8, kind="ExternalInput").ap()
    w4_d = nc.dram_tensor("w4", [128, nch * H], _FP32, kind="ExternalInput").ap()
    w4h_d = nc.dram_tensor("w4h", [128, nch * H], _FP16, kind="ExternalInput").ap()
    out_d = nc.dram_tensor("out", [rpc, D], _FP32, kind="ExternalOutput").ap()

    # split AllGather: each core's first hb row-blocks go in AG-A, the rest
    # in AG-B; the pair permutation puts AG-A src blocks in the first chunks
    xallA = nc.dram_tensor(
        "xallA", [cfg.n_cores, 128, HA * D], _FP16, addr_space="Shared"
    ).ap()
    xallB = nc.dram_tensor(
        "xallB", [cfg.n_cores, 128, (bpc - HA) * D], _FP16, addr_space="Shared"
    ).ap()
    sliceA = nc.dram_tensor("sliceA", [128, HA * D], _FP16).ap()
    sliceB = nc.dram_tensor("sliceB", [128, (bpc - HA) * D], _FP16).ap()
    warm_i = nc.dram_tensor("warm_i", [64], _FP32).ap()
    warm_o = nc.dram_tensor(
        "warm_o", [cfg.n_cores, 64], _FP32, addr_space="Shared"
    ).ap()

    groups = [list(range(cfg.n_cores))]

    def allgather(in_ap, out_ap):
        nc.gpsimd.collective_compute(
            "AllGather",
            mybir.AluOpType.bypass,
            replica_groups=groups,
            ins=[in_ap],
            outs=[out_ap],
        )

    # wave order: phase A = first cb/2 chunks (AG-A src blocks) of every dst
    # block, then phase B.  (b, wv) for wave index i in this order:
    worder = [
        (b, (wppA if ph else 0) + wv)
        for ph in range(2)
        for b in range(bpc)
        for wv in range(wppA if ph == 0 else wppB)
    ]

    with tile.TileContext(nc) as tc:
        with (
            tc.tile_pool(name="tables", bufs=1) as tp,
            tc.tile_pool(name="xsb", bufs=2) as xsbp,
            tc.tile_pool(name="xgw", bufs=4) as xgwp,
            tc.tile_pool(name="acc", bufs=1) as accp,
            tc.tile_pool(name="stage", bufs=2) as stp,
            tc.tile_pool(name="psg", bufs=4, space="PSUM") as ppg,
            tc.tile_pool(name="psyal", bufs=1, space="PSUM") as ppyal,
            tc.tile_pool(name="psyah", bufs=1, space="PSUM") as ppyah,
            tc.tile_pool(name="psybl", bufs=1, space="PSUM") as ppybl,
            tc.tile_pool(name="psybh", bufs=1, space="PSUM") as ppybh,
        ):
            # warmup collective: aligns the cores' CC streams while tables load
            allgather(warm_i, warm_o)

            gsl_sb = tp.tile([128, nslots], _FP8)
            sca_sb = tp.tile([128, nslots], _FP8)
            w4_sb = tp.tile([128, nch, H], _FP32)
            result = accp.tile([128, bpc, D], _FP32)
            xsb1 = xsbp.tile([128, NB, D], _FP16, name="xsb1", tag="xsb")

            # iteration-1 working set first: the first gathers need x and the
            # first table block, not the whole 13 MB of tables
            nc.sync.dma_start(
                out=xsb1[:], in_=xin16.rearrange("p (b f) -> p b f", f=D)
            )
            nc.sync.dma_start(out=w4_sb[:].rearrange("p c h -> p (c h)"), in_=w4_d)
            # Taylor accumulator starts at the identity term (this core's x0).
            nc.sync.dma_start(
                out=result[:],
                in_=x0s_d.rearrange("(j p) f -> p j f", p=128),
            )
            # split the big table loads by (phase, block) in iteration-1
            # wave order, so each gather/scatter wave only waits for the
            # table slice it actually reads
            hcols = (cb // 2) * 128
            for ph in range(2):
                for b in range(bpc):
                    o = b * 2 * hcols + ph * hcols
                    nc.sync.dma_start(
                        out=gsl_sb[:, o : o + hcols], in_=gsl_d[:, o : o + hcols]
                    )
                    nc.sync.dma_start(
                        out=sca_sb[:, o : o + hcols], in_=sca_d[:, o : o + hcols]
                    )

            def gather_wave(b, wv, xsb, psG):
                for ci in range(WCH):
                    c = wv * WCH + ci
                    for (a, wd, j, s_blk) in pieces[c]:
                        gofs = b * NB * ps + s_blk
                        nc.tensor.matmul(
                            psG[a : a + wd, ci, :],
                            lhsT=gsl_sb[:, gofs : gofs + wd],
                            rhs=xsb[:, j, :],
                            start=True,
                            stop=True,
                            tile_position=(0, a),
                        )

            # phase- and half-local accumulators: a PSUM bank may hold only
            # ONE open accumulation group at a time (start=True clears
            # has_written bank-wide), and evacuation reads must not share a
            # bank with in-flight scatter writes (fatal PSUM collision /
            # forced serialization).  4 separate banks: (phase A|B) x
            # (blocks 0-3 | blocks 4-7).
            def scatter_wave(b, wv, psY4, xgw):
                ph = 0 if wv < wppA else 1
                half = 0 if b < HA else 1
                tgt = psY4[ph][half]
                c_lo = 0 if ph == 0 else wppA * WCH
                c_hi = (wppA * WCH - 1) if ph == 0 else cb - 1
                for ci in range(WCH):
                    c = wv * WCH + ci
                    gc = b * cb + c
                    nc.tensor.matmul(
                        tgt[:, b if b < HA else b - HA, :],
                        lhsT=sca_sb[:, gc * 128 : (gc + 1) * 128],
                        rhs=xgw[:, ci, :],
                        start=(c == c_lo),
                        stop=(c == c_hi),
                    )

            for it in range(1, K_TAYLOR + 1):
                coef = 1.0 / math.factorial(it)
                if it == 1:
                    xsb = xsb1
                else:
                    xsb = xsbp.tile([128, NB, D], _FP16, tag="xsb")
                    dst_ap = xsb[:].rearrange("p (k b) f -> p k b f", b=bpc)
                    # A then B: phase-A gathers depend only on AG-A's output
                    # issue x loads from the (mostly idle) scalar engine's DMA
                    # queue so they dispatch as soon as their AllGather lands,
                    # instead of queueing behind this iteration's slice-outs
                    # on the sync engine's queue
                    for (blo_h, bhi_h), xall_h in (((0, HA), xallA), ((HA, bpc), xallB)):
                        src = xall_h.rearrange("k p (b f) -> p k b f", f=D)
                        for g in range(2):
                            k4 = slice(g * 4, (g + 1) * 4)
                            nc.scalar.dma_start(
                                out=dst_ap[:, k4, blo_h:bhi_h, :],
                                in_=src[:, k4, :, :],
                            )

                xnext = stp.tile([128, bpc, D], _FP16, tag="xnext")
                ysum = stp.tile([128, bpc, D], _FP32, name="ysum", tag="ysum")
                psYAl = ppyal.tile([128, bpc, D], _FP32, name="psYAl", tag="psYAl")
                psYAh = ppyah.tile([128, bpc, D], _FP32, name="psYAh", tag="psYAh")
                psYBl = ppybl.tile([128, bpc, D], _FP32, name="psYBl", tag="psYBl")
                psYBh = ppybh.tile([128, bpc, D], _FP32, name="psYBh", tag="psYBh")
                psY4 = ((psYAl, psYAh), (psYBl, psYBh))
                tiles = {}

                def weight_mul(i):
                    b, wv = worder[i]
                    psG, _ = tiles[i]
                    xgw = xgwp.tile([128, WCH, D], _FP16, tag="xgw")
                    tiles[i] = (psG, xgw)
                    gc0 = b * cb + wv * WCH
                    xgw4 = xgw[:].rearrange("p c (h f) -> p c h f", h=H)
                    psG4 = psG[:].rearrange("p c (h f) -> p c h f", h=H)
                    w4v = (
                        w4_sb[:, gc0 : gc0 + WCH, :]
                        .unsqueeze(3)
                        .to_broadcast([128, WCH, H, d])
                    )
                    nc.vector.tensor_mul(xgw4, psG4, w4v)

                def evac_half(hf):
                    # y = psYA + psYB for blocks [0,HA) or [HA,bpc)
                    blo, bhi = (0, HA) if hf == 0 else (HA, bpc)
                    sl = slice(0, bhi - blo)
                    nc.scalar.copy(ysum[:, blo:bhi, :], psY4[0][hf][:, sl, :])
                    nc.vector.scalar_tensor_tensor(
                        ysum[:, blo:bhi, :],
                        psY4[1][hf][:, sl, :],
                        1.0,
                        ysum[:, blo:bhi, :],
                        op0=mybir.AluOpType.mult,
                        op1=mybir.AluOpType.add,
                    )
                    nc.scalar.copy(xnext[:, blo:bhi, :], ysum[:, blo:bhi, :])
                    nc.vector.scalar_tensor_tensor(
                        result[:, blo:bhi, :],
                        ysum[:, blo:bhi, :],
                        coef,
                        result[:, blo:bhi, :],
                        op0=mybir.AluOpType.mult,
                        op1=mybir.AluOpType.add,
                    )
                    if it < K_TAYLOR:
                        sl_d, xall_h = (sliceA, xallA) if hf == 0 else (sliceB, xallB)
                        nc.sync.dma_start(
                            out=sl_d,
                            in_=xnext[:, blo:bhi, :].rearrange("p b f -> p (b f)"),
                        )
                        allgather(sl_d, xall_h)

                # software-pipelined wave loop: gathers run LAG waves ahead of
                # scatters so the PE never stalls on the weight multiply
                nwv = len(worder)
                for i in range(nwv + LAG):
                    if i < nwv:
                        psG = ppg.tile([128, WCH, D], _FP32, tag="psG")
                        tiles[i] = (psG, None)
                        b, wv = worder[i]
                        gather_wave(b, wv, xsb, psG)
                        weight_mul(i)
                    if i >= LAG:
                        b, wv = worder[i - LAG]
                        scatter_wave(b, wv, psY4, tiles.pop(i - LAG)[1])
                        if wv == waves_pb - 1 and b in (HA - 1, bpc - 1):
                            # half (blocks 0:HA or HA:bpc) fully scattered
                            evac_half(0 if b == HA - 1 else 1)

            nc.sync.dma_start(
                out=out_d.rearrange("(j p) f -> p j f", p=128),
                in_=result[:],
            )

    nc.compile()
    return nc


# ------------------------------------------------------------------ driver

_CACHE = {}


def _get_program(cfg: Cfg, R: int):
    key = (cfg, R)
    if key not in _CACHE:
        _CACHE[key] = _build_program(cfg, R)
    return _CACHE[key]


def _in_maps(x0, tables, cfg: Cfg):
    import ml_dtypes

    rpc = cfg.rows_per_core
    # partition-major: xin16[p, B*D + f] = x0[B*128 + p, f]
    x16 = np.ascontiguousarray(
        x0.astype(np.float16)
        .reshape(NB, 128, D)
        .transpose(1, 0, 2)
        .reshape(128, NB * D)
    )
    return [
        {
            "xin16": x16,
            "x0s": np.ascontiguousarray(x0[k * rpc : (k + 1) * rpc]),
            "gsl": t["gsl"],
            "sca": t["sca"],
            "w4": t["w4"],
            "w4h": t["w4h"],
        }
        for k, t in enumerate(tables)
    ]


def run(h, e, src, dst, cfg: Cfg = Cfg(), trace: bool = False):
    """Full pipeline: preprocess, build/compile (cached), execute, assemble."""
    h = np.asarray(h, dtype=np.float32)
    e = np.asarray(e, dtype=np.float32)
    src = np.asarray(src)
    dst = np.asarray(dst)
    nheads = e.shape[0]
    n = h.shape[0]
    dd = h.shape[1] // nheads
    assert (n, nheads, dd) == (cfg.n, H, d), (n, nheads, dd)

    tables, R = _make_tables(e, src, dst, cfg)
    x0 = np.ascontiguousarray(
        h.reshape(nheads, n, dd).transpose(1, 0, 2).reshape(n, nheads * dd)
    )
    nc = _get_program(cfg, R)
    res = run_bass_kernel_spmd(
        nc,
        _in_maps(x0, tables, cfg),
        list(range(cfg.n_cores)),
        trace=trace,
    )
    out = np.concatenate(
        [res.results[k]["out"] for k in range(cfg.n_cores)], axis=0
    )
    # back to reference layout: (n, H, d) node-major -> (H, n, d) -> (N, D)
    out = np.ascontiguousarray(out.reshape(n, nheads, dd).transpose(1, 0, 2)).reshape(
        n, nheads * dd
    )
    return out, res


def kernel(h, e, src, dst):
    out, _ = run(h, e, src, dst)
    return out


# revision 42
# speedup vs baseline: 1.0511x; 1.0511x over previous
"""Trainium2 Bass kernel for nn_LinearDiffusion (truncated Taylor expm(a) @ x).

Math: a = row-normalized symmetric scatter of per-head edge weights onto an
(H, N, N) zero tensor; result = sum_{i=0..6} a^i x / i! with x = h reshaped
per-head.

Strategy (8 NeuronCores, one chip):
  * Sparse formulation, dst-row sharded: core k owns rows [k*1024,(k+1)*1024).
  * x (8192 x 64, all heads interleaved per node row) lives in SBUF as fp16
    (1 MB); no DMA row-gather at all.  Both the gather x[src] and the
    scatter into dst rows run on the Tensor engine as one-hot fp8 matmuls:
      - per (dst-block I, src-block J) pair one full 128-slot chunk, so every
        stationary is a full 128-col fp8 load (FWL-eligible, no column
        tiling, no PE array mode switches - these were worth 2.2x);
        gather: psG[chunk] = G_pair^T @ x_J
      - VectorE: per-head weight multiply, PSUM -> fp16 SBUF
      - scatter: psY_I += S_chunk^T @ xgw_chunk  (fp8 one-hot stationary)
  * The slot layout is identical on every core, so one SPMD program serves
    all 8 cores; only the table *data* (one-hot columns, weights) differs.
  * PSUM discipline: start=True clears has_written bank-wide, and evac
    reads must not share a bank with in-flight matmul writes, so the y
    accumulators are 4 separate banks: (phase A|B) x (blocks 0-3|4-7).
  * Between the k=6 matmuls: two half fp16 AllGathers (each core's blocks
    0-3, then 4-7).  The pair permutation puts AG-A's src blocks in each
    dst block's first chunks, and waves run phase-major, so AG-A overlaps
    the producer's second half and AG-B overlaps the consumer's first.
  * Software-pipelined wave loop: gathers run LAG=2 waves ahead of
    scatters; table loads are split per (phase, block) in first-use order.
"""

import math
from dataclasses import dataclass

import numpy as np

import concourse.bass as bass  # noqa: F401  (kept for callers)
import concourse.tile as tile
from concourse import bacc, mybir
from concourse.bass_utils import run_bass_kernel_spmd

# ----------------------------------------------------------------- config

N, H, E, D = 8192, 4, 131072, 64
d = D // H
NCORES = 8
BLK = 128  # dst/src block size == PE width
NB = N // BLK  # 64 src blocks
K_TAYLOR = 6


@dataclass(frozen=True)
class Cfg:
    n: int = N
    n_cores: int = NCORES
    hi_lo_split: bool = True  # unused (fp16 path is exact enough); kept for test.py

    @property
    def rows_per_core(self):
        return self.n // self.n_cores

    @property
    def blocks_per_core(self):
        return self.rows_per_core // BLK


# ----------------------------------------------------------- preprocessing


def _entries(e, src, dst, n):
    """Unique symmetric entries with 'last write wins' duplicate semantics,
    matching jax's .at[].set() on CPU. Returns (rows, cols, w[H, nnz])."""
    src = src.astype(np.int64)
    dst = dst.astype(np.int64)
    n_edges = len(src)
    keys = np.concatenate([src * n + dst, dst * n + src])
    eid = np.concatenate([np.arange(n_edges), np.arange(n_edges)])
    order = np.arange(2 * n_edges)
    perm = np.lexsort((-order, keys))
    k_sorted = keys[perm]
    first = np.ones(len(k_sorted), dtype=bool)
    first[1:] = k_sorted[1:] != k_sorted[:-1]
    win = perm[first]
    ukeys = k_sorted[first]
    rows = (ukeys // n).astype(np.int64)
    cols = (ukeys % n).astype(np.int64)
    weids = eid[win]
    vals = e[:, weids].astype(np.float64)  # (H, nnz)
    nheads = e.shape[0]
    rowsum = np.zeros((nheads, n), dtype=np.float64)
    for hh in range(nheads):
        rowsum[hh] = np.bincount(rows, weights=vals[hh], minlength=n)
    w = (vals / rowsum[:, rows]).astype(np.float32)
    return rows, cols, w


# Pair order within each dst block: src blocks delivered by the first
# half-AllGather (j % bpc < bpc/2, i.e. every core's first 4 row-blocks)
# come first, so the next iteration's first chunks depend only on AG-A.
def _pair_perm():
    return sorted(range(NB), key=lambda j: (j % 8 >= 4, j))


def _pieces_for_ps(ps):
    """Slot layout: pair rank r (in _pair_perm order) owns slots
    [r*ps, (r+1)*ps) of its dst block. ps is a multiple of 128, so each
    chunk belongs to exactly one pair: full-width FWL-eligible matmuls."""
    assert ps % 128 == 0
    perm = _pair_perm()
    cb = (NB * ps) // 128  # chunks per dst block
    cpp = ps // 128  # chunks per pair
    pieces = [[(0, 128, perm[c // cpp], c * 128)] for c in range(cb)]
    return pieces, cb


def _make_tables(e, src, dst, cfg: Cfg):
    """Per-core device tables. Returns (tables, R) where tables is a list
    over cores of dicts with keys gsl, sca (fp8), w4 (fp32)."""
    import ml_dtypes

    n = cfg.n
    rows, cols, w = _entries(e, src, dst, n)
    nheads = w.shape[0]
    bpc = cfg.blocks_per_core

    # global R so the program structure is identical on every core.
    # R is a multiple of 4 => every pair occupies whole 128-slot chunks, so
    # every gather matmul loads a full 128-col stationary (FWL-eligible, no
    # column tiling, no PE array mode switches).
    cnt = np.zeros((n // BLK, NB), dtype=np.int64)
    np.add.at(cnt, (rows // BLK, cols // BLK), 1)
    R = 4 * int(np.ceil(cnt.max() / 128))
    ps = 32 * R
    cb = (NB * ps) // 128  # chunks per dst block
    nch = bpc * cb
    nslots = nch * 128

    tables = []
    for k in range(cfg.n_cores):
        sel = (rows >= k * 1024) & (rows < (k + 1) * 1024)
        r_k, c_k, w_k = rows[sel], cols[sel], w[:, sel]
        rank = np.empty(NB, dtype=np.int64)
        rank[np.array(_pair_perm())] = np.arange(NB)
        b_loc = r_k // BLK - k * bpc  # 0..7
        key = b_loc * NB + rank[c_k // BLK]
        order = np.argsort(key, kind="stable")
        r_k, c_k, w_k, key = r_k[order], c_k[order], w_k[:, order], key[order]
        grp_start = np.searchsorted(key, key)  # first index of each group
        off_in_pair = np.arange(len(key)) - grp_start
        assert off_in_pair.max(initial=0) < ps
        slot = key * ps + off_in_pair  # key = b_loc * NB + pair_rank

        gsl = np.zeros((128, nslots), dtype=ml_dtypes.float8_e4m3fn)
        sca = np.zeros((128, nslots), dtype=ml_dtypes.float8_e4m3fn)
        w4 = np.zeros((128, nch, nheads), dtype=np.float32)
        gsl[c_k % BLK, slot] = 1.0
        sca[slot % 128, (slot // 128) * 128 + (r_k % BLK)] = 1.0
        w4[slot % 128, slot // 128, :] = w_k.T
        tables.append(
            {
                "gsl": gsl,
                "sca": sca,
                "w4": np.ascontiguousarray(w4.reshape(128, nch * nheads)),
                "w4h": np.ascontiguousarray(
                    w4.reshape(128, nch * nheads).astype(np.float16)
                ),
            }
        )
    return tables, R


# ------------------------------------------------------------ bass program

_FP32 = mybir.dt.float32
_FP16 = mybir.dt.float16
_FP8 = mybir.dt.float8e4


def _build_program(cfg: Cfg, R: int):
    bpc = cfg.blocks_per_core
    rpc = cfg.rows_per_core
    ps = 32 * R
    pieces, cb = _pieces_for_ps(ps)
    nch = bpc * cb
    nslots = nch * 128

    WCH = 8  # chunks per wave (one wave = 1 PSUM bank of gathered rows)
    LAG = 2  # scatter waves trail gather waves by this many steps
    assert cb % (2 * WCH) == 0
    waves_pb = cb // WCH  # waves per dst block
    wpp = waves_pb // 2  # waves per (block, phase)
    hb = bpc // 2  # blocks per half-AllGather

    nc = bacc.Bacc(
        "TRN2",
        target_bir_lowering=False,
        debug=False,
        num_devices=cfg.n_cores,
    )

    # partition-major x layouts: [core, p, b, f] so SBUF loads are contiguous
    xin16 = nc.dram_tensor("xin16", [128, NB * D], _FP16, kind="ExternalInput").ap()
    x0s_d = nc.dram_tensor("x0s", [rpc, D], _FP32, kind="ExternalInput").ap()
    gsl_d = nc.dram_tensor("gsl", [128, nslots], _FP8, kind="ExternalInput").ap()
    sca_d = nc.dram_tensor("sca", [128, nslots], _FP8, kind="ExternalInput").ap()
    w4_d = nc.dram_tensor("w4", [128, nch * H], _FP32, kind="ExternalInput").ap()
    w4h_d = nc.dram_tensor("w4h", [128, nch * H], _FP16, kind="ExternalInput").ap()
    out_d = nc.dram_tensor("out", [rpc, D], _FP32, kind="ExternalOutput").ap()

    # split AllGather: each core's first hb row-blocks go in AG-A, the rest
    # in AG-B; the pair permutation puts AG-A src blocks in the first chunks
    xallA = nc.dram_tensor(
        "xallA", [cfg.n_cores, 128, hb * D], _FP16, addr_space="Shared"
    ).ap()
    xallB = nc.dram_tensor(
        "xallB", [cfg.n_cores, 128, hb * D], _FP16, addr_space="Shared"
    ).ap()
    sliceA = nc.dram_tensor("sliceA", [128, hb * D], _FP16).ap()
    sliceB = nc.dram_tensor("sliceB", [128, hb * D], _FP16).ap()
    warm_i = nc.dram_tensor("warm_i", [64], _FP32).ap()
    warm_o = nc.dram_tensor(
        "warm_o", [cfg.n_cores, 64], _FP32, addr_space="Shared"
    ).ap()

    groups = [list(range(cfg.n_cores))]

    def allgather(in_ap, out_ap):
        nc.gpsimd.collective_compute(
            "AllGather",
            mybir.AluOpType.bypass,
            replica_groups=groups,
            ins=[in_ap],
            outs=[out_ap],
        )

    # wave order: phase A = first cb/2 chunks (AG-A src blocks) of every dst
    # block, then phase B.  (b, wv) for wave index i in this order:
    worder = [
        (b, ph * wpp + wv)
        for ph in range(2)
        for b in range(bpc)
        for wv in range(wpp)
    ]

    with tile.TileContext(nc) as tc:
        with (
            tc.tile_pool(name="tables", bufs=1) as tp,
            tc.tile_pool(name="xsb", bufs=2) as xsbp,
            tc.tile_pool(name="xgw", bufs=4) as xgwp,
            tc.tile_pool(name="acc", bufs=1) as accp,
            tc.tile_pool(name="stage", bufs=2) as stp,
            tc.tile_pool(name="psg", bufs=4, space="PSUM") as ppg,
            tc.tile_pool(name="psyal", bufs=1, space="PSUM") as ppyal,
            tc.tile_pool(name="psyah", bufs=1, space="PSUM") as ppyah,
            tc.tile_pool(name="psybl", bufs=1, space="PSUM") as ppybl,
            tc.tile_pool(name="psybh", bufs=1, space="PSUM") as ppybh,
        ):
            # warmup collective: aligns the cores' CC streams while tables load
            allgather(warm_i, warm_o)

            gsl_sb = tp.tile([128, nslots], _FP8)
            sca_sb = tp.tile([128, nslots], _FP8)
            w4_sb = tp.tile([128, nch, H], _FP32)
            result = accp.tile([128, bpc, D], _FP32)
            xsb1 = xsbp.tile([128, NB, D], _FP16, name="xsb1", tag="xsb")

            # iteration-1 working set first: the first gathers need x and the
            # first table block, not the whole 13 MB of tables
            nc.sync.dma_start(
                out=xsb1[:], in_=xin16.rearrange("p (b f) -> p b f", f=D)
            )
            nc.sync.dma_start(out=w4_sb[:].rearrange("p c h -> p (c h)"), in_=w4_d)
            # Taylor accumulator starts at the identity term (this core's x0).
            nc.sync.dma_start(
                out=result[:],
                in_=x0s_d.rearrange("(j p) f -> p j f", p=128),
            )
            # split the big table loads by (phase, block) in iteration-1
            # wave order, so each gather/scatter wave only waits for the
            # table slice it actually reads
            hcols = (cb // 2) * 128
            for ph in range(2):
                for b in range(bpc):
                    o = b * 2 * hcols + ph * hcols
                    nc.sync.dma_start(
                        out=gsl_sb[:, o : o + hcols], in_=gsl_d[:, o : o + hcols]
                    )
                    nc.sync.dma_start(
                        out=sca_sb[:, o : o + hcols], in_=sca_d[:, o : o + hcols]
                    )

            def gather_wave(b, wv, xsb, psG):
                for ci in range(WCH):
                    c = wv * WCH + ci
                    for (a, wd, j, s_blk) in pieces[c]:
                        gofs = b * NB * ps + s_blk
                        nc.tensor.matmul(
                            psG[a : a + wd, ci, :],
                            lhsT=gsl_sb[:, gofs : gofs + wd],
                            rhs=xsb[:, j, :],
                            start=True,
                            stop=True,
                            tile_position=(0, a),
                        )

            # phase- and half-local accumulators: a PSUM bank may hold only
            # ONE open accumulation group at a time (start=True clears
            # has_written bank-wide), and evacuation reads must not share a
            # bank with in-flight scatter writes (fatal PSUM collision /
            # forced serialization).  4 separate banks: (phase A|B) x
            # (blocks 0-3 | blocks 4-7).
            def scatter_wave(b, wv, psY4, xgw):
                ph = wv // wpp
                tgt = psY4[ph][b // hb]
                c_lo, c_hi = ph * wpp * WCH, (ph + 1) * wpp * WCH - 1
                for ci in range(WCH):
                    c = wv * WCH + ci
                    gc = b * cb + c
                    nc.tensor.matmul(
                        tgt[:, b % hb, :],
                        lhsT=sca_sb[:, gc * 128 : (gc + 1) * 128],
                        rhs=xgw[:, ci, :],
                        start=(c == c_lo),
                        stop=(c == c_hi),
                    )

            for it in range(1, K_TAYLOR + 1):
                coef = 1.0 / math.factorial(it)
                if it == 1:
                    xsb = xsb1
                else:
                    xsb = xsbp.tile([128, NB, D], _FP16, tag="xsb")
                    dst_ap = xsb[:].rearrange("p (k b) f -> p k b f", b=bpc)
                    # A then B: phase-A gathers depend only on AG-A's output
                    # issue x loads from the (mostly idle) scalar engine's DMA
                    # queue so they dispatch as soon as their AllGather lands,
                    # instead of queueing behind this iteration's slice-outs
                    # on the sync engine's queue
                    for half, xall_h in ((0, xallA), (1, xallB)):
                        src = xall_h.rearrange("k p (b f) -> p k b f", f=D)
                        for g in range(4):
                            k2 = slice(g * 2, (g + 1) * 2)
                            nc.scalar.dma_start(
                                out=dst_ap[:, k2, half * hb : (half + 1) * hb, :],
                                in_=src[:, k2, :, :],
                            )

                xnext = stp.tile([128, bpc, D], _FP16, tag="xnext")
                ysum = stp.tile([128, bpc, D], _FP32, name="ysum", tag="ysum")
                psYAl = ppyal.tile([128, bpc, D], _FP32, name="psYAl", tag="psYAl")
                psYAh = ppyah.tile([128, bpc, D], _FP32, name="psYAh", tag="psYAh")
                psYBl = ppybl.tile([128, bpc, D], _FP32, name="psYBl", tag="psYBl")
                psYBh = ppybh.tile([128, bpc, D], _FP32, name="psYBh", tag="psYBh")
                psY4 = ((psYAl, psYAh), (psYBl, psYBh))
                tiles = {}

                def weight_mul(i):
                    b, wv = worder[i]
                    psG, _ = tiles[i]
                    xgw = xgwp.tile([128, WCH, D], _FP16, tag="xgw")
                    tiles[i] = (psG, xgw)
                    gc0 = b * cb + wv * WCH
                    xgw4 = xgw[:].rearrange("p c (h f) -> p c h f", h=H)
                    psG4 = psG[:].rearrange("p c (h f) -> p c h f", h=H)
                    w4v = (
                        w4_sb[:, gc0 : gc0 + WCH, :]
                        .unsqueeze(3)
                        .to_broadcast([128, WCH, H, d])
                    )
                    nc.vector.tensor_mul(xgw4, psG4, w4v)

                def evac_half(hf):
                    # y = psYA + psYB for blocks [hf*hb, (hf+1)*hb)
                    blo, bhi = hf * hb, (hf + 1) * hb
                    sl = slice(0, hb)
                    nc.scalar.copy(ysum[:, blo:bhi, :], psY4[0][hf][:, sl, :])
                    nc.vector.scalar_tensor_tensor(
                        ysum[:, blo:bhi, :],
                        psY4[1][hf][:, sl, :],
                        1.0,
                        ysum[:, blo:bhi, :],
                        op0=mybir.AluOpType.mult,
                        op1=mybir.AluOpType.add,
                    )
                    nc.scalar.copy(xnext[:, blo:bhi, :], ysum[:, blo:bhi, :])
                    if it < K_TAYLOR:
                        sl_d, xall_h = (sliceA, xallA) if hf == 0 else (sliceB, xallB)
                        nc.sync.dma_start(
                            out=sl_d,
                            in_=xnext[:, blo:bhi, :].rearrange("p b f -> p (b f)"),
                        )
                        allgather(sl_d, xall_h)
                    # Taylor accumulation off the AllGather critical chain
                    nc.vector.scalar_tensor_tensor(
                        result[:, blo:bhi, :],
                        ysum[:, blo:bhi, :],
                        coef,
                        result[:, blo:bhi, :],
                        op0=mybir.AluOpType.mult,
                        op1=mybir.AluOpType.add,
                    )

                # software-pipelined wave loop: gathers run LAG waves ahead of
                # scatters so the PE never stalls on the weight multiply
                nwv = len(worder)
                for i in range(nwv + LAG):
                    if i < nwv:
                        psG = ppg.tile([128, WCH, D], _FP32, tag="psG")
                        tiles[i] = (psG, None)
                        b, wv = worder[i]
                        gather_wave(b, wv, xsb, psG)
                        weight_mul(i)
                    if i >= LAG:
                        b, wv = worder[i - LAG]
                        scatter_wave(b, wv, psY4, tiles.pop(i - LAG)[1])
                        if wv == waves_pb - 1 and b % hb == hb - 1:
                            # half hf (blocks 0-3 or 4-7) fully scattered
                            evac_half(b // hb)

            nc.sync.dma_start(
                out=out_d.rearrange("(j p) f -> p j f", p=128),
                in_=result[:],
            )

    nc.compile()
    return nc


# ------------------------------------------------------------------ driver

_CACHE = {}


def _get_program(cfg: Cfg, R: int):
    key = (cfg, R)
    if key not in _CACHE:
        _CACHE[key] = _build_program(cfg, R)
    return _CACHE[key]


def _in_maps(x0, tables, cfg: Cfg):
    import ml_dtypes

    rpc = cfg.rows_per_core
    # partition-major: xin16[p, B*D + f] = x0[B*128 + p, f]
    x16 = np.ascontiguousarray(
        x0.astype(np.float16)
        .reshape(NB, 128, D)
        .transpose(1, 0, 2)
        .reshape(128, NB * D)
    )
    return [
        {
            "xin16": x16,
            "x0s": np.ascontiguousarray(x0[k * rpc : (k + 1) * rpc]),
            "gsl": t["gsl"],
            "sca": t["sca"],
            "w4": t["w4"],
            "w4h": t["w4h"],
        }
        for k, t in enumerate(tables)
    ]


def run(h, e, src, dst, cfg: Cfg = Cfg(), trace: bool = False):
    """Full pipeline: preprocess, build/compile (cached), execute, assemble."""
    h = np.asarray(h, dtype=np.float32)
    e = np.asarray(e, dtype=np.float32)
    src = np.asarray(src)
    dst = np.asarray(dst)
    nheads = e.shape[0]
    n = h.shape[0]
    dd = h.shape[1] // nheads
    assert (n, nheads, dd) == (cfg.n, H, d), (n, nheads, dd)

    tables, R = _make_tables(e, src, dst, cfg)
    x0 = np.ascontiguousarray(
        h.reshape(nheads, n, dd).transpose(1, 0, 2).reshape(n, nheads * dd)
    )
    nc = _get_program(cfg, R)
    res = run_bass_kernel_spmd(
        nc,
        _in_maps(x0, tables, cfg),
        list(range(cfg.n_cores)),
        trace=trace,
    )
    out = np.concatenate(
        [res.results[k]["out"] for k in range(cfg.n_cores)], axis=0
    )
    # back to reference layout: (n, H, d) node-major -> (H, n, d) -> (N, D)
    out = np.ascontiguousarray(out.reshape(n, nheads, dd).transpose(1, 0, 2)).reshape(
        n, nheads * dd
    )
    return out, res


def kernel(h, e, src, dst):
    out, _ = run(h, e, src, dst)
    return out
